# revision 2
# baseline (speedup 1.0000x reference)
"""2D DWT (db4, circular pad, stride-2) forward on 8 Trainium2 NeuronCores.

v2 strategy (pure data parallel, 12 images of 512x512 per core), pure-fp16
datapath: the harness tolerance is 2e-2 and fp16 end-to-end lands ~7e-4,
so x streams in as fp16, each separable filter pass is a SINGLE fp16
banded matmul accumulating in fp32 PSUM, coefficients round to fp16 in
SBUF and stream out as fp16 (the host upcasts). Per-core HBM traffic is
2 B/pixel each way (~12.6 MB -> ~35 us at 358 GB/s); TensorE streams
~536 cols per 512-col accumulation group (~24 us total). DMA-bound.

  stage 1 (filter along H):  V[w, (hj,a)]   = sum_h  X[h, w] * M[h, (hj,a)]
  stage 2 (filter along W):  out[hj,(wj,b)] = sum_w  V[w, a*256+hj] * M[w, (wj,b)]

M is the 512x512 interleaved filter-bank matrix M[i, 2j+f] = dec[f][(i-2j)%512]
(8 nonzeros per column). Each 128-row chunk of M only has ~67 nonzero j
columns, so each PSUM accumulation group streams just the banded column
slices (~536 of 2048 columns per bank) instead of dense 512-wide matmuls.
X chunks are the stationary operands (4 LDWEIGHTS per group, FWL-eligible),
so the partition dim rotates h -> w -> hj with no transposes anywhere.

I/O layouts keep every DMA line contiguous per partition: x arrives
host-swizzled as (img, p, (c, w)) [4 KiB/partition/image]; the output
leaves as (img, hjc, p, (s, wj)) [2 KiB lines] and the host unshuffles.
"""

import sys

sys.path.insert(0, "/opt/trn_rl_repo")

import numpy as np

L = 512
NJ = L // 2  # 256
TAPS = 8
N_CORES = 8
IMGS_PER_CORE = 12  # 32 batch * 3 channels / 8 cores

_compiled = {}


def _build_M(dec: np.ndarray) -> np.ndarray:
    """M[i, 2*j + f] = dec[f][(i - 2j) mod 512]; filters interleaved so each
    128-row chunk's nonzero columns form one contiguous range (plus wrap)."""
    M = np.zeros((L, L), dtype=np.float32)
    i = np.arange(L)[:, None]
    j = np.arange(NJ)[None, :]
    k = (i - 2 * j) % L
    mask = k < TAPS
    for f in range(2):
        M[:, f::2] = np.where(mask, np.asarray(dec[f])[np.minimum(k, TAPS - 1)], 0.0)
    return M


def _col_slices(c: int):
    """Interleaved nonzero column ranges of M rows [128c, 128c+128):
    j in [64c-3, 64c+63] (mod 256) -> interleaved cols [2j, 2j+1]."""
    lo_j, hi_j = 64 * c - 3, 64 * c + 63
    if lo_j < 0:
        return [(0, 2 * (hi_j + 1)), (2 * (lo_j % NJ), 2 * NJ)]
    return [(2 * lo_j, 2 * (hi_j + 1))]


def _group_mms():
    """(chunk, c0, c1) matmul slices for one PSUM accumulation group,
    big slices around the tiny N=6 wrap slice so its LDWEIGHTS exposure
    hides behind long streams (LDW pipelines ~2 deep)."""
    mms = [(c, c0, c1) for c in range(4) for (c0, c1) in _col_slices(c)]
    mms.sort(key=lambda m: -(m[2] - m[1]))
    # [134, 134, 6, 134, 128]
    mms[2], mms[4] = mms[4], mms[2]
    return mms


def _build_nc():
    import concourse.bass as bass  # noqa: F401
    import concourse.tile as tile
    from concourse import bacc, mybir

    f32 = mybir.dt.float32
    f16 = mybir.dt.float16
    nc = bacc.Bacc("TRN2", target_bir_lowering=False, debug=False,
                   num_devices=N_CORES)
    xh_d = nc.dram_tensor("xh", [IMGS_PER_CORE, 128, 4 * L], f16,
                          kind="ExternalInput")
    mh_d = nc.dram_tensor("mh", [L, L], f16, kind="ExternalInput")
    o_d = nc.dram_tensor("out", [IMGS_PER_CORE, 2, 128, 4 * NJ], f16,
                         kind="ExternalOutput")

    with tile.TileContext(nc) as tc:
        with (
            tc.tile_pool(name="mpool", bufs=1) as mpool,
            tc.tile_pool(name="xpool", bufs=3) as xpool,
            tc.tile_pool(name="vpool", bufs=2) as vpool,
            tc.tile_pool(name="opool", bufs=3) as opool,
            tc.tile_pool(name="pvpool", bufs=4, space="PSUM") as pvpool,
            tc.tile_pool(name="popool", bufs=4, space="PSUM") as popool,
        ):
            # M: 4 h-chunks side by side -> (128, 4*512) fp16
            mth = mpool.tile([128, 4 * L], f16, tag="mth")
            nc.sync.dma_start(
                mth[:].rearrange("p (c w) -> p c w", c=4),
                mh_d[:].rearrange("(c p) w -> p c w", p=128),
            )

            for img in range(IMGS_PER_CORE):
                # image: 4 h-chunks side by side -> (128, 4*512) fp16,
                # contiguous per partition in DRAM (host pre-swizzled)
                xht = xpool.tile([128, 4 * L], f16, tag="xht")
                nc.sync.dma_start(xht[:], xh_d[img])

                # stage 1: V[w, (hj,a)], w-chunk wc in v cols [512wc, 512wc+512),
                # de-interleaved: [0:256) = a=0 (lo), [256:512) = a=1 (hi)
                vht = vpool.tile([128, 4 * L], f16, tag="vht")
                for wc in range(4):
                    pv = pvpool.tile([128, L], f32, tag="pv")
                    mms = _group_mms()
                    for n, (hc, c0, c1) in enumerate(mms):
                        nc.tensor.matmul(
                            pv[:, c0:c1],
                            xht[:, L * hc + 128 * wc : L * hc + 128 * wc + 128],
                            mth[:, L * hc + c0 : L * hc + c1],
                            start=(n == 0),
                            stop=(n == len(mms) - 1),
                        )
                    # de-interleave + fp32 -> fp16 (DVE)
                    for f in range(2):
                        dst = slice(L * wc + NJ * f, L * wc + NJ * f + NJ)
                        nc.vector.tensor_copy(vht[:, dst], pv[:, f : L : 2])

                # stage 2: per (hjc, a) one PSUM bank of out[hj, (wj,b)].
                # ot cols = (hjc, s, wj); subband s = a + 2b.
                ot = opool.tile([128, 2 * 4 * NJ], f16, tag="ot")
                for hjc in range(2):
                    for a in range(2):
                        po = popool.tile([128, L], f32, tag="po")
                        mms = _group_mms()
                        off = NJ * a + 128 * hjc
                        for n, (wc, c0, c1) in enumerate(mms):
                            nc.tensor.matmul(
                                po[:, c0:c1],
                                vht[:, L * wc + off : L * wc + off + 128],
                                mth[:, L * wc + c0 : L * wc + c1],
                                start=(n == 0),
                                stop=(n == len(mms) - 1),
                            )
                        base = 4 * NJ * hjc
                        # scalar handles 3 of 4 (hjc,a) groups, DVE the last:
                        # balances ~153 vs ~245 Gelem/s engine rates.
                        eng = nc.vector.tensor_copy if (hjc, a) == (1, 1) \
                            else nc.scalar.copy
                        eng(ot[:, base + NJ * a : base + NJ * a + NJ],
                            po[:, 0 : L : 2])
                        eng(ot[:, base + NJ * (2 + a) : base + NJ * (2 + a) + NJ],
                            po[:, 1 : L : 2])
                nc.sync.dma_start(
                    o_d[img].rearrange("h p f -> p h f"),
                    ot[:].rearrange("p (h f) -> p h f", h=2),
                )

    nc.finalize()
    return nc


def _in_maps(x: np.ndarray, dec: np.ndarray) -> list[dict]:
    mh = _build_M(dec).astype(np.float16)
    # (96, h, w) -> (96, p, c, w): partition p holds rows {p, 128+p, 256+p, 384+p}
    xh = (x.reshape(96, 4, 128, L).swapaxes(1, 2)
           .astype(np.float16).reshape(96, 128, 4 * L))
    return [
        {
            "xh": xh[IMGS_PER_CORE * c : IMGS_PER_CORE * (c + 1)],
            "mh": mh,
        }
        for c in range(N_CORES)
    ]


def kernel(x: np.ndarray, dec: np.ndarray) -> np.ndarray:
    from concourse.bass_utils import run_bass_kernel_spmd

    x = np.ascontiguousarray(np.asarray(x, dtype=np.float32))
    dec = np.asarray(dec, dtype=np.float32)
    B, C, H, W = x.shape
    assert (B, C, H, W) == (32, 3, 512, 512) and dec.shape == (2, 8)

    if "nc" not in _compiled:
        _compiled["nc"] = _build_nc()
    nc = _compiled["nc"]

    in_maps = _in_maps(x, dec)
    res = run_bass_kernel_spmd(nc, in_maps, list(range(N_CORES))).results
    o = np.concatenate([r["out"] for r in res], axis=0)  # (96, 2, 128, 1024) f16
    # (img, hjc, p, s, w) -> (img, s, hjc*128+p, w), upcast
    o = o.reshape(96, 2, 128, 4, NJ).transpose(0, 3, 1, 2, 4)
    return np.ascontiguousarray(o, dtype=np.float32).reshape(B, C * 4, NJ, NJ)


# revision 3
# speedup vs baseline: 1.4872x; 1.4872x over previous
"""2D DWT (db4, circular pad, stride-2) forward on 8 Trainium2 NeuronCores.

v3 (pure data parallel, 12 images of 512x512 per core), fp16 datapath:
  stage 1 (filter along H):  V[w, (hj,a)]   = sum_h  X[h, w] * M[h, (hj,a)]
  stage 2 (filter along W):  out[hj,(wj,b)] = sum_w  V[w, a*256+hj] * M[w, (wj,b)]

vs v2:
- M is stored COMPACT: each 128-row chunk keeps only its ~134 nonzero
  banded columns (M is 8-banded) -> [128, 4*136] instead of [128, 2048].
- PSUM tiles span 2 banks; each is filled by two 5-matmul accumulation
  groups, then drained by ONE merged 1024-col copy that simultaneously
  de-interleaves (hj,a)->(a,hj) via a 4-D access pattern. Halves the
  per-instruction overhead of the PSUM->SBUF path (the v2 bottleneck).
- Stage-1 drains on DVE (fp16 V for the stage-2 stationary operand),
  stage-2 drains on ScalarE; input DMAs ride the SP HWDGE ring, output
  DMAs the Activation HWDGE ring, so the two streams never queue behind
  each other.
- PE program order is software-pipelined: stage-2 of image i is emitted
  after stage-1 of image i+1, hiding the drain latency.

Output leaves as (img, hjc, p, (a, b, wj)) fp16, host unshuffles+upcasts.
"""

import sys

sys.path.insert(0, "/opt/trn_rl_repo")

import numpy as np

L = 512
NJ = L // 2  # 256
TAPS = 8
N_CORES = 8
IMGS_PER_CORE = 12  # 32 batch * 3 channels / 8 cores
MC = 136  # compact M columns per chunk (134 used + 2 pad)

_compiled = {}


def _build_M_compact(dec: np.ndarray) -> np.ndarray:
    """Compact banded filter matrix, chunk-major, partition-contiguous:
    Mc[p, c*136 + 2*t + f] = dec[f][(128c + p - 2*j_c(t)) mod 512] where
    j_c(t) = (64c - 3 + t) mod 256, t in [0, 67). Returns (128, 4*136)."""
    dec = np.asarray(dec, dtype=np.float32)
    Mc = np.zeros((128, 4 * MC), dtype=np.float32)
    p = np.arange(128)[:, None]
    t = np.arange(67)
    for c in range(4):
        j = (64 * c - 3 + t) % NJ
        k = (128 * c + p - 2 * j) % L
        mask = k < TAPS
        for f in range(2):
            Mc[:, c * MC + 2 * t + f] = np.where(
                mask, dec[f][np.minimum(k, TAPS - 1)], 0.0
            )
    return Mc


def _group_mms():
    """(chunk, rt_c0, rt_c1, out_c0, out_c1) slices for one accumulation
    group. Chunk c's compact cols [0,134) are j = 64c-3 .. 64c+63 (mod 256)
    interleaved; chunk 0 wraps: cols [0,6) -> out [506,512), [6,134) ->
    out [0,128). Order: big slices around the small wrap slice."""
    return [
        (1, 0, 134, 122, 256),
        (2, 0, 134, 250, 384),
        (0, 0, 6, 506, 512),
        (3, 0, 134, 378, 512),
        (0, 6, 134, 0, 128),
    ]


def _build_nc():
    import concourse.bass as bass  # noqa: F401
    import concourse.tile as tile
    from concourse import bacc, mybir

    f32 = mybir.dt.float32
    f16 = mybir.dt.float16
    nc = bacc.Bacc("TRN2", target_bir_lowering=False, debug=False,
                   num_devices=N_CORES)
    xh_d = nc.dram_tensor("xh", [IMGS_PER_CORE, 128, 4 * L], f16,
                          kind="ExternalInput")
    mh_d = nc.dram_tensor("mh", [128, 4 * MC], f16, kind="ExternalInput")
    o_d = nc.dram_tensor("out", [IMGS_PER_CORE, 2, 128, 4 * NJ], f16,
                         kind="ExternalOutput")

    with tile.TileContext(nc) as tc:
        with (
            tc.tile_pool(name="mpool", bufs=1) as mpool,
            tc.tile_pool(name="xpool", bufs=4) as xpool,
            tc.tile_pool(name="vpool", bufs=2) as vpool,
            tc.tile_pool(name="opool", bufs=2) as opool,
            tc.tile_pool(name="pvpool", bufs=2, space="PSUM") as pvpool,
            tc.tile_pool(name="popool", bufs=2, space="PSUM") as popool,
        ):
            mth = mpool.tile([128, 4 * MC], f16, tag="mth")
            nc.sync.dma_start(mth[:], mh_d[:])

            vhts, ots = {}, {}
            for step in range(IMGS_PER_CORE + 1):
                if step < IMGS_PER_CORE:
                    img = step
                    xht = xpool.tile([128, 4 * L], f16, tag="xht")
                    nc.sync.dma_start(xht[:], xh_d[img])
                    # stage 1: vht cols (wc, a, hj); lt = x chunk stationary
                    vht = vpool.tile([128, 4 * L], f16, tag="vht")
                    vhts[img] = vht
                    for wcp in range(2):
                        pv = pvpool.tile([128, 2 * L], f32, tag="pv")
                        for half in range(2):
                            wc = 2 * wcp + half
                            o0 = L * half
                            for n, (hc, r0, r1, c0, c1) in enumerate(_group_mms()):
                                nc.tensor.matmul(
                                    pv[:, o0 + c0 : o0 + c1],
                                    xht[:, L * hc + 128 * wc : L * hc + 128 * wc + 128],
                                    mth[:, MC * hc + r0 : MC * hc + r1],
                                    start=(n == 0),
                                    stop=(n == 4),
                                )
                        # merged drain: (wc, hj, a) -> (wc, a, hj), fp16
                        nc.vector.tensor_copy(
                            vht[:, 1024 * wcp : 1024 * wcp + 1024].rearrange(
                                "p (c a h) -> p c a h", c=2, a=2),
                            pv[:].rearrange("p (c h a) -> p c a h", c=2, a=2),
                        )
                if step >= 1:
                    img = step - 1
                    vht = vhts.pop(img)
                    # stage 2: ot cols (hjc, a, b, wj); lt = V slice stationary
                    ot = opool.tile([128, 2 * 4 * NJ], f16, tag="ot")
                    for hjc in range(2):
                        po = popool.tile([128, 2 * L], f32, tag="po")
                        for a in range(2):
                            o0 = L * a
                            off = 256 * a + 128 * hjc
                            for n, (wc, r0, r1, c0, c1) in enumerate(_group_mms()):
                                nc.tensor.matmul(
                                    po[:, o0 + c0 : o0 + c1],
                                    vht[:, L * wc + off : L * wc + off + 128],
                                    mth[:, MC * wc + r0 : MC * wc + r1],
                                    start=(n == 0),
                                    stop=(n == 4),
                                )
                        # merged drain: (a, wj, b) -> (a, b, wj), fp16
                        nc.scalar.copy(
                            ot[:, 1024 * hjc : 1024 * hjc + 1024].rearrange(
                                "p (c b w) -> p c b w", c=2, b=2),
                            po[:].rearrange("p (c w b) -> p c b w", c=2, b=2),
                        )
                    nc.scalar.dma_start(
                        o_d[img].rearrange("h p f -> p h f"),
                        ot[:].rearrange("p (h f) -> p h f", h=2),
                    )

    nc.finalize()
    return nc


def _in_maps(x: np.ndarray, dec: np.ndarray) -> list[dict]:
    mh = _build_M_compact(dec).astype(np.float16)
    # (96, h, w) -> (96, p, c, w): partition p holds rows {p, 128+p, ...}
    xh = (x.reshape(96, 4, 128, L).swapaxes(1, 2)
           .astype(np.float16).reshape(96, 128, 4 * L))
    return [
        {
            "xh": xh[IMGS_PER_CORE * c : IMGS_PER_CORE * (c + 1)],
            "mh": mh,
        }
        for c in range(N_CORES)
    ]


def kernel(x: np.ndarray, dec: np.ndarray) -> np.ndarray:
    from concourse.bass_utils import run_bass_kernel_spmd

    x = np.ascontiguousarray(np.asarray(x, dtype=np.float32))
    dec = np.asarray(dec, dtype=np.float32)
    B, C, H, W = x.shape
    assert (B, C, H, W) == (32, 3, 512, 512) and dec.shape == (2, 8)

    if "nc" not in _compiled:
        _compiled["nc"] = _build_nc()
    nc = _compiled["nc"]

    in_maps = _in_maps(x, dec)
    res = run_bass_kernel_spmd(nc, in_maps, list(range(N_CORES))).results
    o = np.concatenate([r["out"] for r in res], axis=0)  # (96, 2, 128, 1024)
    # (img, hjc, p, a, b, wj) -> (img, s=2b+a... wait s=a+2b -> order (b,a))
    o = o.reshape(96, 2, 128, 2, 2, NJ).transpose(0, 4, 3, 1, 2, 5)
    return np.ascontiguousarray(o, dtype=np.float32).reshape(B, C * 4, NJ, NJ)


# revision 6
# speedup vs baseline: 1.6100x; 1.0826x over previous
"""2D DWT (db4, circular pad, stride-2) forward on 8 Trainium2 NeuronCores.

v3 (pure data parallel, 12 images of 512x512 per core), fp16 datapath:
  stage 1 (filter along H):  V[w, (hj,a)]   = sum_h  X[h, w] * M[h, (hj,a)]
  stage 2 (filter along W):  out[hj,(wj,b)] = sum_w  V[w, a*256+hj] * M[w, (wj,b)]

vs v2:
- M is stored COMPACT: each 128-row chunk keeps only its ~134 nonzero
  banded columns (M is 8-banded) -> [128, 4*136] instead of [128, 2048].
- PSUM tiles span 2 banks; each is filled by two 5-matmul accumulation
  groups, then drained by ONE merged 1024-col copy that simultaneously
  de-interleaves (hj,a)->(a,hj) via a 4-D access pattern. Halves the
  per-instruction overhead of the PSUM->SBUF path (the v2 bottleneck).
- Stage-1 drains on DVE (fp16 V for the stage-2 stationary operand),
  stage-2 drains on ScalarE; input DMAs ride the SP HWDGE ring, output
  DMAs the Activation HWDGE ring, so the two streams never queue behind
  each other.
- PE program order is software-pipelined: stage-2 of image i is emitted
  after stage-1 of image i+1, hiding the drain latency.

Output leaves as (img, hjc, p, (a, b, wj)) fp16, host unshuffles+upcasts.
"""

import sys

sys.path.insert(0, "/opt/trn_rl_repo")

import numpy as np

L = 512
NJ = L // 2  # 256
TAPS = 8
N_CORES = 8
IMGS_PER_CORE = 12  # 32 batch * 3 channels / 8 cores
MC = 136  # compact M columns per chunk (134 used + 2 pad)

_compiled = {}


def _build_M_compact(dec: np.ndarray) -> np.ndarray:
    """Compact banded filter matrix, chunk-major, partition-contiguous:
    Mc[p, c*136 + 2*t + f] = dec[f][(128c + p - 2*j_c(t)) mod 512] where
    j_c(t) = (64c - 3 + t) mod 256, t in [0, 67). Returns (128, 4*136)."""
    dec = np.asarray(dec, dtype=np.float32)
    Mc = np.zeros((128, 4 * MC), dtype=np.float32)
    p = np.arange(128)[:, None]
    t = np.arange(67)
    for c in range(4):
        j = (64 * c - 3 + t) % NJ
        k = (128 * c + p - 2 * j) % L
        mask = k < TAPS
        for f in range(2):
            Mc[:, c * MC + 2 * t + f] = np.where(
                mask, dec[f][np.minimum(k, TAPS - 1)], 0.0
            )
    return Mc


def _group_mms():
    """(chunk, rt_c0, rt_c1, out_c0, out_c1) slices for one accumulation
    group. Chunk c's compact cols [0,134) are j = 64c-3 .. 64c+63 (mod 256)
    interleaved; chunk 0 wraps: cols [0,6) -> out [506,512), [6,134) ->
    out [0,128). Order: big slices around the small wrap slice."""
    return [
        (1, 0, 134, 122, 256),
        (2, 0, 134, 250, 384),
        (0, 0, 6, 506, 512),
        (3, 0, 134, 378, 512),
        (0, 6, 134, 0, 128),
    ]


def _build_nc():
    import concourse.bass as bass  # noqa: F401
    import concourse.tile as tile
    from concourse import bacc, mybir

    f32 = mybir.dt.float32
    f16 = mybir.dt.float16
    nc = bacc.Bacc("TRN2", target_bir_lowering=False, debug=False,
                   num_devices=N_CORES)
    xh_d = nc.dram_tensor("xh", [IMGS_PER_CORE, 128, 4 * L], f16,
                          kind="ExternalInput")
    mh_d = nc.dram_tensor("mh", [128, 4 * MC], f16, kind="ExternalInput")
    o_d = nc.dram_tensor("out", [IMGS_PER_CORE, 2, 128, 4 * NJ], f16,
                         kind="ExternalOutput")

    with tile.TileContext(nc) as tc:
        with (
            tc.tile_pool(name="mpool", bufs=1) as mpool,
            tc.tile_pool(name="xpool", bufs=6) as xpool,
            tc.tile_pool(name="vpool", bufs=2) as vpool,
            tc.tile_pool(name="opool", bufs=4) as opool,
            tc.tile_pool(name="pvpool", bufs=2, space="PSUM") as pvpool,
            tc.tile_pool(name="popool", bufs=2, space="PSUM") as popool,
        ):
            mth = mpool.tile([128, 4 * MC], f16, tag="mth")
            nc.sync.dma_start(mth[:], mh_d[:])

            vhts = {}
            for img in range(IMGS_PER_CORE):
                xht = xpool.tile([128, 4 * L], f16, tag="xht")
                nc.sync.dma_start(xht[:], xh_d[img])
                # stage 1: vht cols (wc, a, hj); lt = x chunk stationary
                vht = vpool.tile([128, 4 * L], f16, tag="vht")
                vhts[img] = vht
                for wcp in range(2):
                    pv = pvpool.tile([128, 2 * L], f32, tag="pv")
                    for half in range(2):
                        wc = 2 * wcp + half
                        o0 = L * half
                        for n, (hc, r0, r1, c0, c1) in enumerate(_group_mms()):
                            nc.tensor.matmul(
                                pv[:, o0 + c0 : o0 + c1],
                                xht[:, L * hc + 128 * wc : L * hc + 128 * wc + 128],
                                mth[:, MC * hc + r0 : MC * hc + r1],
                                start=(n == 0),
                                stop=(n == 4),
                            )
                    # merged drain: (wc, hj, a) -> (wc, a, hj), fp16;
                    # the last pair drains in two halves so stage 2's
                    # first group can start one bank earlier
                    if wcp == 0:
                        nc.vector.tensor_copy(
                            vht[:, 0:1024].rearrange(
                                "p (c a h) -> p c a h", c=2, a=2),
                            pv[:].rearrange("p (c h a) -> p c a h", c=2, a=2),
                        )
                    else:
                        for half in range(2):
                            nc.vector.tensor_copy(
                                vht[:, 1024 + 512 * half : 1536 + 512 * half]
                                .rearrange("p (a h) -> p a h", a=2),
                                pv[:, 512 * half : 512 * half + 512]
                                .rearrange("p (h a) -> p a h", a=2),
                            )
                # stage 2: ot cols (hjc, a, b, wj); lt = V slice stationary
                ot = opool.tile([128, 2 * 4 * NJ], f16, tag="ot")
                for hjc in range(2):
                    po = popool.tile([128, 2 * L], f32, tag="po")
                    for a in range(2):
                        o0 = L * a
                        off = 256 * a + 128 * hjc
                        for n, (wc, r0, r1, c0, c1) in enumerate(_group_mms()):
                            nc.tensor.matmul(
                                po[:, o0 + c0 : o0 + c1],
                                vht[:, L * wc + off : L * wc + off + 128],
                                mth[:, MC * wc + r0 : MC * wc + r1],
                                start=(n == 0),
                                stop=(n == 4),
                            )
                    # merged drain: (a, wj, b) -> (a, b, wj), fp16
                    nc.scalar.copy(
                        ot[:, 1024 * hjc : 1024 * hjc + 1024].rearrange(
                            "p (c b w) -> p c b w", c=2, b=2),
                        po[:].rearrange("p (c w b) -> p c b w", c=2, b=2),
                    )
                nc.scalar.dma_start(
                    o_d[img].rearrange("h p f -> p h f"),
                    ot[:].rearrange("p (h f) -> p h f", h=2),
                )

    nc.finalize()
    return nc


def _in_maps(x: np.ndarray, dec: np.ndarray) -> list[dict]:
    mh = _build_M_compact(dec).astype(np.float16)
    # (96, h, w) -> (96, p, c, w): partition p holds rows {p, 128+p, ...}
    xh = (x.reshape(96, 4, 128, L).swapaxes(1, 2)
           .astype(np.float16).reshape(96, 128, 4 * L))
    return [
        {
            "xh": xh[IMGS_PER_CORE * c : IMGS_PER_CORE * (c + 1)],
            "mh": mh,
        }
        for c in range(N_CORES)
    ]


def kernel(x: np.ndarray, dec: np.ndarray) -> np.ndarray:
    from concourse.bass_utils import run_bass_kernel_spmd

    x = np.ascontiguousarray(np.asarray(x, dtype=np.float32))
    dec = np.asarray(dec, dtype=np.float32)
    B, C, H, W = x.shape
    assert (B, C, H, W) == (32, 3, 512, 512) and dec.shape == (2, 8)

    if "nc" not in _compiled:
        _compiled["nc"] = _build_nc()
    nc = _compiled["nc"]

    in_maps = _in_maps(x, dec)
    res = run_bass_kernel_spmd(nc, in_maps, list(range(N_CORES))).results
    o = np.concatenate([r["out"] for r in res], axis=0)  # (96, 2, 128, 1024)
    # (img, hjc, p, a, b, wj) -> (img, s=2b+a... wait s=a+2b -> order (b,a))
    o = o.reshape(96, 2, 128, 2, 2, NJ).transpose(0, 4, 3, 1, 2, 5)
    return np.ascontiguousarray(o, dtype=np.float32).reshape(B, C * 4, NJ, NJ)
